# revision 1
# baseline (speedup 1.0000x reference)
"""GNN message passing (gather + segment-sum) on 8 TRN2 NeuronCores.

Strategy (edge-parallel with node-partitioned output; no collectives):
  - Host: bucket edges by (core = dst // 6250, src-half, dst-window-of-128).
    Core c owns output rows [c*6250, (c+1)*6250) so partial sums ARE final --
    no all-reduce needed.  Within a core, edges are grouped by 128-node dst
    windows; each group is padded to a multiple of 128 edges (common tile
    counts across all 8 cores so one SPMD program serves every core).
  - Device, per core:
      * bulk `dma_gather` of x[src] rows (fp16 table, 256B/row) from HBM into
        SBUF, in big chunks (HW-accelerated SWDGE gather; int16 indices, so
        the table is addressed as two halves: rows [0,32768) and [32768,50000)).
      * per 128-edge tile, build one-hot S[e, n] = (dst_local[e] == n) on the
        DVE with a broadcast `is_equal` against an iota row constant.
      * matmul S^T @ G accumulated in PSUM per 128-node window: the PE does
        the segment reduction.  PSUM (f32) -> SBUF accumulator -> HBM out.
  - Host: concatenate the 8 per-core [6250, 128] slices.

The one-hot/matmul trick makes the scatter-add race-free and keeps HBM
traffic at the roofline: ~21 MB of gathered rows per core dominates.
"""

import os
import numpy as np

N = 50000          # nodes
D = 128            # feature dim
C = 8              # cores
E_TOT = 640000     # edges (any count works; hardcoded shapes only use N, D)
NLOC = N // C      # 6250 output rows per core
P = 128
N_WIN = (NLOC + P - 1) // P        # 49 windows of 128 dst nodes per core
NLOC_PAD = N_WIN * P               # 6272 (padded output rows per core)
SPLIT = 32768                      # int16 gather-index limit
SENT = 300.0                       # dst sentinel for padded edges (never matches iota 0..127)
CHUNK_TILES = 4                    # 128-edge tiles per dma_gather call (512 idx = 33
                                   # ring slots/lane, so ~3 calls pipeline in the
                                   # 128-desc SWDGE ring; >=2048 idx/call overflows it)

LAST_RESULT = None                 # BassKernelResults of the most recent run (for test.py)

_prog_cache = {}


def _ensure_ntff_hook():
    """Provide antenv.axon_hooks (missing from this image) so
    run_bass_kernel_spmd(trace=True) under axon can capture NTFF profiles.
    Harmless no-op when tracing is off or pieces are unavailable."""
    import sys
    import types
    try:
        import antenv.axon_hooks  # noqa: F401
        return
    except ImportError:
        pass
    try:
        import antenv
        mod = types.ModuleType("antenv.axon_hooks")
        mod._hook = None
        mod.set_axon_ntff_profile_hook = lambda h: setattr(mod, "_hook", h)
        mod.get_axon_ntff_profile_hook = lambda: mod._hook
        sys.modules["antenv.axon_hooks"] = mod
        antenv.axon_hooks = mod
        from trn_agent_boot.trn_boot import _ntff_profile_via_ctypes
        so_path = "/opt/axon/libaxon_pjrt.so"
        if os.path.exists(so_path):
            mod.set_axon_ntff_profile_hook(_ntff_profile_via_ctypes(so_path))
    except Exception:
        pass


def _host_prep(x, edge_index):
    """Bucket + pad edges; build per-core device input arrays."""
    x = np.asarray(x, dtype=np.float32)
    ei = np.asarray(edge_index)
    src = ei[0].astype(np.int64)
    dst = ei[1].astype(np.int64)
    E = src.shape[0]

    core = dst // NLOC
    dloc = dst - core * NLOC
    win = dloc >> 7                 # dst window within core
    pcol = dloc & 127               # dst node within window
    half = (src >= SPLIT).astype(np.int64)

    # counts[c, h, w]
    counts = np.zeros((C, 2, N_WIN), np.int64)
    np.add.at(counts, (core, half, win), 1)
    # common (max-over-cores) tile counts so one SPMD program fits all cores
    T = (-(-counts // P)).max(axis=0)        # [2, N_WIN] tiles per (half, window)
    T[0] = np.maximum(T[0], 1)               # lo pass initializes every window's acc

    L = T.sum(axis=1) * P                    # padded edges per half
    tile_base = np.zeros((2, N_WIN), np.int64)
    tile_base[0, 1:] = np.cumsum(T[0])[:-1]
    tile_base[1, 1:] = np.cumsum(T[1])[:-1]

    # sort edges by (core, half, window); stable order within groups is fine
    order = np.lexsort((win, half, core))
    s_src = src[order]
    s_p = pcol[order]
    gsz = counts.reshape(-1)
    gstart = np.zeros(C * 2 * N_WIN + 1, np.int64)
    np.cumsum(gsz, out=gstart[1:])

    xh = np.ascontiguousarray(x.astype(np.float16))
    iota = np.tile(np.arange(P, dtype=np.float16)[None, :], (P, 1))

    def wrap_idx(a):  # int16 [L] -> [128, L//16] (16-part wrap, replicated x8)
        w16 = np.ascontiguousarray(a.reshape(-1, 16).T)
        return np.ascontiguousarray(np.tile(w16, (8, 1)))

    per_core = []
    for c in range(C):
        srcs = [np.zeros(L[0], np.int16), np.zeros(L[1], np.int16)]
        dstp = [np.full(L[0], SENT, np.float16), np.full(L[1], SENT, np.float16)]
        for h in range(2):
            for w in range(N_WIN):
                g = (c * 2 + h) * N_WIN + w
                a, b = gstart[g], gstart[g + 1]
                n = b - a
                if n == 0:
                    continue
                pos = tile_base[h, w] * P
                adj = 0 if h == 0 else SPLIT
                srcs[h][pos:pos + n] = (s_src[a:b] - adj).astype(np.int16)
                dstp[h][pos:pos + n] = s_p[a:b].astype(np.float16)
        dstp_all = np.concatenate(dstp)                       # [L0 + L1]
        dstp_tile = np.ascontiguousarray(dstp_all.reshape(-1, P).T)  # [128, T_tot]
        meta = np.concatenate([dstp_tile, iota], axis=1)      # [128, T_tot + 128]
        idx_all = np.concatenate(srcs)                        # [L0 + L1]
        m = {
            "xh": xh,
            "idx": wrap_idx(idx_all),
            "meta": np.ascontiguousarray(meta),
        }
        per_core.append(m)

    return per_core, tuple(T[0]), tuple(T[1]), int(L[0]), int(L[1])


def _build_program(T_lo, T_hi, L_lo, L_hi):
    import concourse.bass as bass
    import concourse.tile as tile
    import concourse.mybir as mybir
    from concourse import bacc

    dt = mybir.dt
    nc = bacc.Bacc("TRN2", target_bir_lowering=False, debug=False, num_devices=C)

    xh = nc.dram_tensor("xh", [N, D], dt.float16, kind="ExternalInput")
    L_tot = L_lo + L_hi
    idx_d = nc.dram_tensor("idx", [128, L_tot // 16], dt.int16, kind="ExternalInput")
    T_tot = L_tot // P
    meta_d = nc.dram_tensor("meta", [128, T_tot + 128], dt.float16, kind="ExternalInput")
    out_d = nc.dram_tensor("out", [NLOC_PAD, D], dt.float32, kind="ExternalOutput")

    with tile.TileContext(nc) as tc:
        with (
            tc.tile_pool(name="metap", bufs=1) as metap,
            tc.tile_pool(name="gp", bufs=3) as gpool,
            tc.tile_pool(name="sp", bufs=6) as spool,
            tc.tile_pool(name="pp", bufs=4, space="PSUM") as ppool,
            tc.tile_pool(name="accp", bufs=1) as accp,
        ):
            idx_t = metap.tile([128, L_tot // 16], dt.int16, tag="idx", name="idx_t")
            nc.sync.dma_start(idx_t[:], idx_d[:])
            meta_t = metap.tile([128, T_tot + 128], dt.float16, tag="meta", name="meta_t")
            nc.sync.dma_start(meta_t[:], meta_d[:])
            dstp_t = meta_t[:, :T_tot]
            iota_t = meta_t[:, T_tot:]

            acc = accp.tile([128, N_WIN * P], dt.float32, tag="acc")

            gt = 0  # global tile index (column into dstp_t)
            for h in range(2):
                Th = T_lo if h == 0 else T_hi
                total_tiles = sum(Th)
                if total_tiles == 0:
                    continue
                src_view = xh[:SPLIT] if h == 0 else xh[SPLIT:]
                icol0 = 0 if h == 0 else L_lo // 16   # column base into idx_t
                th = 0        # tile index within this half
                G = None
                ntc = 0       # tiles in current chunk
                for wi in range(N_WIN):
                    tw = Th[wi]
                    if tw == 0:
                        continue
                    pt = ppool.tile([128, 128], dt.float32, tag="psum")
                    for t in range(tw):
                        cslot = th % CHUNK_TILES
                        if cslot == 0:
                            ntc = min(CHUNK_TILES, total_tiles - th)
                            G = gpool.tile([128, ntc * 128], dt.float16, tag="gather")
                            nidx = ntc * 128
                            nc.gpsimd.dma_gather(
                                G[:].rearrange("p (t f) -> p t f", f=128),
                                src_view,
                                idx_t[:, icol0 + th * 8:icol0 + (th + ntc) * 8],
                                nidx,
                                nidx,
                                D,
                            )
                        S = spool.tile([128, 128], dt.float16, tag="sel")
                        nc.vector.tensor_tensor(
                            out=S[:],
                            in0=dstp_t[:, gt:gt + 1].to_broadcast([128, 128]),
                            in1=iota_t[:],
                            op=mybir.AluOpType.is_equal,
                        )
                        nc.tensor.matmul(
                            pt[:],
                            S[:],
                            G[:, cslot * 128:(cslot + 1) * 128],
                            start=(t == 0),
                            stop=(t == tw - 1),
                        )
                        th += 1
                        gt += 1
                    lo, hi = wi * 128, (wi + 1) * 128
                    if h == 0:
                        nc.vector.tensor_copy(acc[:, lo:hi], pt[:])
                    else:
                        nc.vector.tensor_add(acc[:, lo:hi], acc[:, lo:hi], pt[:])
                    last_touch = (h == 1) or (T_hi[wi] == 0)
                    if last_touch:
                        nc.sync.dma_start(out_d[lo:hi, :], acc[:, lo:hi])
    nc.compile()
    return nc


def kernel(x, edge_index):
    global LAST_RESULT
    _ensure_ntff_hook()
    from concourse.bass_utils import run_bass_kernel_spmd

    per_core, T_lo, T_hi, L_lo, L_hi = _host_prep(x, edge_index)

    key = (T_lo, T_hi)
    if key not in _prog_cache:
        _prog_cache[key] = _build_program(T_lo, T_hi, L_lo, L_hi)
    nc = _prog_cache[key]

    res = run_bass_kernel_spmd(nc, per_core, core_ids=list(range(C)))
    LAST_RESULT = res
    out = np.concatenate([r["out"][:NLOC] for r in res.results], axis=0)
    return out.astype(np.float32)



# revision 4
# speedup vs baseline: 3.1577x; 3.1577x over previous
"""GNN message passing (gather + segment-sum) on 8 TRN2 NeuronCores.

Strategy (edge-parallel with node-partitioned output; no collectives):
  - Host: bucket edges by (core = dst // 6250, src-half, dst-window-of-128).
    Core c owns output rows [c*6250, (c+1)*6250) so partial sums ARE final --
    no all-reduce needed.  Within a core, edges are grouped by 128-node dst
    windows; each group is padded to a multiple of 128 edges (common tile
    counts across all 8 cores so one SPMD program serves every core).
  - Device, per core:
      * bulk `dma_gather` of x[src] rows (fp16 table, 256B/row) from HBM into
        SBUF, in big chunks (HW-accelerated SWDGE gather; int16 indices, so
        the table is addressed as two halves: rows [0,32768) and [32768,50000)).
      * per 128-edge tile, build one-hot S[e, n] = (dst_local[e] == n) on the
        DVE with a broadcast `is_equal` against an iota row constant.
      * matmul S^T @ G accumulated in PSUM per 128-node window: the PE does
        the segment reduction.  PSUM (f32) -> SBUF accumulator -> HBM out.
  - Host: concatenate the 8 per-core [6250, 128] slices.

The one-hot/matmul trick makes the scatter-add race-free and keeps HBM
traffic at the roofline: ~21 MB of gathered rows per core dominates.
"""

import os
import numpy as np

N = 50000          # nodes
D = 128            # feature dim
C = 8              # cores
E_TOT = 640000     # edges (any count works; hardcoded shapes only use N, D)
NLOC = N // C      # 6250 output rows per core
P = 128
N_WIN = (NLOC + P - 1) // P        # 49 windows of 128 dst nodes per core
NLOC_PAD = N_WIN * P               # 6272 (padded output rows per core)
SPLIT = 32768                      # int16 gather-index limit
SENT = 300.0                       # dst sentinel for padded edges (never matches iota 0..127)
CHUNK_TILES = 7                    # 128-edge tiles per dma_gather call (512 idx = 33
                                   # ring slots/lane, so ~3 calls pipeline in the
                                   # 128-desc SWDGE ring; >=2048 idx/call overflows it)

LAST_RESULT = None                 # BassKernelResults of the most recent run (for test.py)

_prog_cache = {}


def _ensure_ntff_hook():
    """Provide antenv.axon_hooks (missing from this image) so
    run_bass_kernel_spmd(trace=True) under axon can capture NTFF profiles.
    Harmless no-op when tracing is off or pieces are unavailable."""
    import sys
    import types
    try:
        import antenv.axon_hooks  # noqa: F401
        return
    except ImportError:
        pass
    try:
        import antenv
        mod = types.ModuleType("antenv.axon_hooks")
        mod._hook = None
        mod.set_axon_ntff_profile_hook = lambda h: setattr(mod, "_hook", h)
        mod.get_axon_ntff_profile_hook = lambda: mod._hook
        sys.modules["antenv.axon_hooks"] = mod
        antenv.axon_hooks = mod
        from trn_agent_boot.trn_boot import _ntff_profile_via_ctypes
        so_path = "/opt/axon/libaxon_pjrt.so"
        if os.path.exists(so_path):
            mod.set_axon_ntff_profile_hook(_ntff_profile_via_ctypes(so_path))
    except Exception:
        pass


def _host_prep(x, edge_index):
    """Bucket + pad edges; build per-core device input arrays."""
    x = np.asarray(x, dtype=np.float32)
    ei = np.asarray(edge_index)
    src = ei[0].astype(np.int64)
    dst = ei[1].astype(np.int64)
    E = src.shape[0]

    core = dst // NLOC
    dloc = dst - core * NLOC
    win = dloc >> 7                 # dst window within core
    pcol = dloc & 127               # dst node within window
    half = (src >= SPLIT).astype(np.int64)

    # counts[c, h, w]
    counts = np.zeros((C, 2, N_WIN), np.int64)
    np.add.at(counts, (core, half, win), 1)
    # common (max-over-cores) tile counts so one SPMD program fits all cores
    T = (-(-counts // P)).max(axis=0)        # [2, N_WIN] tiles per (half, window)
    T[0] = np.maximum(T[0], 1)               # lo pass initializes every window's acc

    L = T.sum(axis=1) * P                    # padded edges per half
    tile_base = np.zeros((2, N_WIN), np.int64)
    tile_base[0, 1:] = np.cumsum(T[0])[:-1]
    tile_base[1, 1:] = np.cumsum(T[1])[:-1]

    # sort edges by (core, half, window); stable order within groups is fine
    order = np.lexsort((win, half, core))
    s_src = src[order]
    s_p = pcol[order]
    gsz = counts.reshape(-1)
    gstart = np.zeros(C * 2 * N_WIN + 1, np.int64)
    np.cumsum(gsz, out=gstart[1:])

    xh = np.ascontiguousarray(x.astype(np.float16))
    iota = np.tile(np.arange(P, dtype=np.float16)[None, :], (P, 1))

    def wrap_idx(a):  # int16 [L] -> [128, L//16] (16-part wrap, replicated x8)
        w16 = np.ascontiguousarray(a.reshape(-1, 16).T)
        return np.ascontiguousarray(np.tile(w16, (8, 1)))

    per_core = []
    for c in range(C):
        srcs = [np.zeros(L[0], np.int16), np.zeros(L[1], np.int16)]
        dstp = [np.full(L[0], SENT, np.float16), np.full(L[1], SENT, np.float16)]
        for h in range(2):
            for w in range(N_WIN):
                g = (c * 2 + h) * N_WIN + w
                a, b = gstart[g], gstart[g + 1]
                n = b - a
                if n == 0:
                    continue
                pos = tile_base[h, w] * P
                adj = 0 if h == 0 else SPLIT
                srcs[h][pos:pos + n] = (s_src[a:b] - adj).astype(np.int16)
                dstp[h][pos:pos + n] = s_p[a:b].astype(np.float16)
        dstp_all = np.concatenate(dstp)                       # [L0 + L1]
        dstp_tile = np.ascontiguousarray(dstp_all.reshape(-1, P).T)  # [128, T_tot]
        meta = np.concatenate([dstp_tile, iota], axis=1)      # [128, T_tot + 128]
        idx_all = np.concatenate(srcs)                        # [L0 + L1]
        m = {
            "xh": xh,
            "idx": wrap_idx(idx_all),
            "meta": np.ascontiguousarray(meta),
        }
        per_core.append(m)

    return per_core, tuple(T[0]), tuple(T[1]), int(L[0]), int(L[1])


def _build_program(T_lo, T_hi, L_lo, L_hi):
    import concourse.bass as bass
    import concourse.tile as tile
    import concourse.mybir as mybir
    from concourse import bacc

    dt = mybir.dt
    nc = bacc.Bacc("TRN2", target_bir_lowering=False, debug=False, num_devices=C)

    xh = nc.dram_tensor("xh", [N, D], dt.float16, kind="ExternalInput")
    L_tot = L_lo + L_hi
    idx_d = nc.dram_tensor("idx", [128, L_tot // 16], dt.int16, kind="ExternalInput")
    T_tot = L_tot // P
    meta_d = nc.dram_tensor("meta", [128, T_tot + 128], dt.float16, kind="ExternalInput")
    out_d = nc.dram_tensor("out", [NLOC_PAD, D], dt.float32, kind="ExternalOutput")

    with tile.TileContext(nc) as tc:
        with (
            tc.tile_pool(name="metap", bufs=1) as metap,
            tc.tile_pool(name="gp", bufs=3) as gpool,
            tc.tile_pool(name="sp", bufs=6) as spool,
            tc.tile_pool(name="pp", bufs=4, space="PSUM") as ppool,
            tc.tile_pool(name="accp", bufs=1) as accp,
        ):
            idx_t = metap.tile([128, L_tot // 16], dt.int16, tag="idx", name="idx_t")
            nc.sync.dma_start(idx_t[:], idx_d[:])
            meta_t = metap.tile([128, T_tot + 128], dt.float16, tag="meta", name="meta_t")
            nc.sync.dma_start(meta_t[:], meta_d[:])
            dstp_t = meta_t[:, :T_tot]
            iota_t = meta_t[:, T_tot:]

            acc = accp.tile([128, N_WIN * P], dt.float32, tag="acc")

            gt = 0  # global tile index (column into dstp_t)
            for h in range(2):
                Th = T_lo if h == 0 else T_hi
                total_tiles = sum(Th)
                if total_tiles == 0:
                    continue
                src_view = xh[:SPLIT] if h == 0 else xh[SPLIT:]
                icol0 = 0 if h == 0 else L_lo // 16   # column base into idx_t
                th = 0        # tile index within this half
                G = None
                ntc = 0       # tiles in current chunk
                for wi in range(N_WIN):
                    tw = Th[wi]
                    if tw == 0:
                        continue
                    pt = ppool.tile([128, 128], dt.float32, tag="psum")
                    for t in range(tw):
                        cslot = th % CHUNK_TILES
                        if cslot == 0:
                            ntc = min(CHUNK_TILES, total_tiles - th)
                            G = gpool.tile([128, ntc * 128], dt.float16, tag="gather")
                            nidx = ntc * 128
                            nc.gpsimd.dma_gather(
                                G[:].rearrange("p (t f) -> p t f", f=128),
                                src_view,
                                idx_t[:, icol0 + th * 8:icol0 + (th + ntc) * 8],
                                nidx,
                                nidx,
                                D,
                            )
                        S = spool.tile([128, 128], dt.float16, tag="sel")
                        nc.vector.tensor_tensor(
                            out=S[:],
                            in0=dstp_t[:, gt:gt + 1].to_broadcast([128, 128]),
                            in1=iota_t[:],
                            op=mybir.AluOpType.is_equal,
                        )
                        nc.tensor.matmul(
                            pt[:],
                            S[:],
                            G[:, cslot * 128:(cslot + 1) * 128],
                            start=(t == 0),
                            stop=(t == tw - 1),
                        )
                        th += 1
                        gt += 1
                    lo, hi = wi * 128, (wi + 1) * 128
                    if h == 0:
                        nc.vector.tensor_copy(acc[:, lo:hi], pt[:])
                    else:
                        nc.vector.tensor_add(acc[:, lo:hi], acc[:, lo:hi], pt[:])
                    last_touch = (h == 1) or (T_hi[wi] == 0)
                    if last_touch:
                        nc.sync.dma_start(out_d[lo:hi, :], acc[:, lo:hi])
    nc.compile()
    return nc


def kernel(x, edge_index):
    global LAST_RESULT
    _ensure_ntff_hook()
    from concourse.bass_utils import run_bass_kernel_spmd

    per_core, T_lo, T_hi, L_lo, L_hi = _host_prep(x, edge_index)

    key = (T_lo, T_hi)
    if key not in _prog_cache:
        _prog_cache[key] = _build_program(T_lo, T_hi, L_lo, L_hi)
    nc = _prog_cache[key]

    res = run_bass_kernel_spmd(nc, per_core, core_ids=list(range(C)))
    LAST_RESULT = res
    out = np.concatenate([r["out"][:NLOC] for r in res.results], axis=0)
    return out.astype(np.float32)



# revision 6
# speedup vs baseline: 5.4346x; 1.7211x over previous
"""GNN message passing (gather + segment-sum) on 8 TRN2 NeuronCores.

Strategy (edge-parallel with node-partitioned output; no collectives):
  - Host: bucket edges by (core = dst // 6250, src-half, dst-window-of-128).
    Core c owns output rows [c*6250, (c+1)*6250) so partial sums ARE final --
    no all-reduce needed.  Within a core, edges are grouped by 128-node dst
    windows; each group is padded to a multiple of 128 edges (common tile
    counts across all 8 cores so one SPMD program serves every core).
  - Device, per core:
      * bulk `dma_gather` of x[src] rows (fp16 table, 256B/row) from HBM into
        SBUF, in big chunks (HW-accelerated SWDGE gather; int16 indices, so
        the table is addressed as two halves: rows [0,32768) and [32768,50000)).
      * per 128-edge tile, build one-hot S[e, n] = (dst_local[e] == n) on the
        DVE with a broadcast `is_equal` against an iota row constant.
      * matmul S^T @ G accumulated in PSUM per 128-node window: the PE does
        the segment reduction.  PSUM (f32) -> SBUF accumulator -> HBM out.
  - Host: concatenate the 8 per-core [6250, 128] slices.

The one-hot/matmul trick makes the scatter-add race-free and keeps HBM
traffic at the roofline: ~21 MB of gathered rows per core dominates.
"""

import os
import numpy as np

N = 50000          # nodes
D = 128            # feature dim
C = 8              # cores
E_TOT = 640000     # edges (any count works; hardcoded shapes only use N, D)
NLOC = N // C      # 6250 output rows per core
P = 128
N_WIN = (NLOC + P - 1) // P        # 49 windows of 128 dst nodes per core
NLOC_PAD = N_WIN * P               # 6272 (padded output rows per core)
SPLIT = 32768                      # int16 gather-index limit
SENT = 300.0                       # dst sentinel for padded edges (never matches iota 0..127)
CHUNK_TILES = 7                    # 128-edge tiles per dma_gather call (512 idx = 33
                                   # ring slots/lane, so ~3 calls pipeline in the
                                   # 128-desc SWDGE ring; >=2048 idx/call overflows it)

LAST_RESULT = None                 # BassKernelResults of the most recent run (for test.py)

_prog_cache = {}


def _ensure_ntff_hook():
    """Provide antenv.axon_hooks (missing from this image) so
    run_bass_kernel_spmd(trace=True) under axon can capture NTFF profiles.
    Harmless no-op when tracing is off or pieces are unavailable."""
    import sys
    import types
    try:
        import antenv.axon_hooks  # noqa: F401
        return
    except ImportError:
        pass
    try:
        import antenv
        mod = types.ModuleType("antenv.axon_hooks")
        mod._hook = None
        mod.set_axon_ntff_profile_hook = lambda h: setattr(mod, "_hook", h)
        mod.get_axon_ntff_profile_hook = lambda: mod._hook
        sys.modules["antenv.axon_hooks"] = mod
        antenv.axon_hooks = mod
        from trn_agent_boot.trn_boot import _ntff_profile_via_ctypes
        so_path = "/opt/axon/libaxon_pjrt.so"
        if os.path.exists(so_path):
            mod.set_axon_ntff_profile_hook(_ntff_profile_via_ctypes(so_path))
    except Exception:
        pass


def _host_prep(x, edge_index):
    """Bucket + pad edges; build per-core device input arrays."""
    x = np.asarray(x, dtype=np.float32)
    ei = np.asarray(edge_index)
    src = ei[0].astype(np.int64)
    dst = ei[1].astype(np.int64)
    E = src.shape[0]

    core = dst // NLOC
    dloc = dst - core * NLOC
    win = dloc >> 7                 # dst window within core
    pcol = dloc & 127               # dst node within window
    half = (src >= SPLIT).astype(np.int64)

    # counts[c, h, w]
    counts = np.zeros((C, 2, N_WIN), np.int64)
    np.add.at(counts, (core, half, win), 1)
    # common (max-over-cores) tile counts so one SPMD program fits all cores
    T = (-(-counts // P)).max(axis=0)        # [2, N_WIN] tiles per (half, window)
    T[0] = np.maximum(T[0], 1)               # lo pass initializes every window's acc

    L = T.sum(axis=1) * P                    # padded edges per half
    tile_base = np.zeros((2, N_WIN), np.int64)
    tile_base[0, 1:] = np.cumsum(T[0])[:-1]
    tile_base[1, 1:] = np.cumsum(T[1])[:-1]

    # sort edges by (core, half, window); stable order within groups is fine
    order = np.lexsort((win, half, core))
    s_src = src[order]
    s_p = pcol[order]
    gsz = counts.reshape(-1)
    gstart = np.zeros(C * 2 * N_WIN + 1, np.int64)
    np.cumsum(gsz, out=gstart[1:])

    xh = np.ascontiguousarray(x.astype(np.float16))
    iota = np.tile(np.arange(P, dtype=np.float16)[None, :], (P, 1))

    def wrap_idx(a):  # int16 [L] -> [128, L//16] (16-part wrap, replicated x8)
        w16 = np.ascontiguousarray(a.reshape(-1, 16).T)
        return np.ascontiguousarray(np.tile(w16, (8, 1)))

    per_core = []
    for c in range(C):
        srcs = [np.zeros(L[0], np.int16), np.zeros(L[1], np.int16)]
        dstp = [np.full(L[0], SENT, np.float16), np.full(L[1], SENT, np.float16)]
        for h in range(2):
            for w in range(N_WIN):
                g = (c * 2 + h) * N_WIN + w
                a, b = gstart[g], gstart[g + 1]
                n = b - a
                if n == 0:
                    continue
                pos = tile_base[h, w] * P
                adj = 0 if h == 0 else SPLIT
                srcs[h][pos:pos + n] = (s_src[a:b] - adj).astype(np.int16)
                dstp[h][pos:pos + n] = s_p[a:b].astype(np.float16)
        dstp_all = np.concatenate(dstp)                       # [L0 + L1]
        dstp_tile = np.ascontiguousarray(dstp_all.reshape(-1, P).T)  # [128, T_tot]
        meta = np.concatenate([dstp_tile, iota], axis=1)      # [128, T_tot + 128]
        idx_all = np.concatenate(srcs)                        # [L0 + L1]
        m = {
            "xh": xh,
            "idx": wrap_idx(idx_all),
            "meta": np.ascontiguousarray(meta),
        }
        per_core.append(m)

    return per_core, tuple(T[0]), tuple(T[1]), int(L[0]), int(L[1])


def _build_program(T_lo, T_hi, L_lo, L_hi):
    import concourse.bass as bass
    import concourse.tile as tile
    import concourse.mybir as mybir
    from concourse import bacc

    dt = mybir.dt
    nc = bacc.Bacc("TRN2", target_bir_lowering=False, debug=False, num_devices=C,
                   num_swdge_queues=2)

    xh = nc.dram_tensor("xh", [N, D], dt.float16, kind="ExternalInput")
    L_tot = L_lo + L_hi
    idx_d = nc.dram_tensor("idx", [128, L_tot // 16], dt.int16, kind="ExternalInput")
    T_tot = L_tot // P
    meta_d = nc.dram_tensor("meta", [128, T_tot + 128], dt.float16, kind="ExternalInput")
    out_d = nc.dram_tensor("out", [NLOC_PAD, D], dt.float32, kind="ExternalOutput")

    with tile.TileContext(nc) as tc:
        with (
            tc.tile_pool(name="metap", bufs=1) as metap,
            tc.tile_pool(name="gp", bufs=3) as gpool,
            tc.tile_pool(name="sp", bufs=6) as spool,
            tc.tile_pool(name="pp", bufs=4, space="PSUM") as ppool,
            tc.tile_pool(name="accp", bufs=1) as accp,
        ):
            idx_t = metap.tile([128, L_tot // 16], dt.int16, tag="idx", name="idx_t")
            nc.sync.dma_start(idx_t[:], idx_d[:])
            meta_t = metap.tile([128, T_tot + 128], dt.float16, tag="meta", name="meta_t")
            nc.sync.dma_start(meta_t[:], meta_d[:])
            dstp_t = meta_t[:, :T_tot]
            iota_t = meta_t[:, T_tot:]

            acc = accp.tile([128, N_WIN * P], dt.float32, tag="acc")

            gt = 0  # global tile index (column into dstp_t)
            for h in range(2):
                Th = T_lo if h == 0 else T_hi
                total_tiles = sum(Th)
                if total_tiles == 0:
                    continue
                src_view = xh[:SPLIT] if h == 0 else xh[SPLIT:]
                icol0 = 0 if h == 0 else L_lo // 16   # column base into idx_t
                th = 0        # tile index within this half
                G = None
                ntc = 0       # tiles in current chunk
                for wi in range(N_WIN):
                    tw = Th[wi]
                    if tw == 0:
                        continue
                    pt = ppool.tile([128, 128], dt.float32, tag="psum")
                    for t in range(tw):
                        cslot = th % CHUNK_TILES
                        if cslot == 0:
                            ntc = min(CHUNK_TILES, total_tiles - th)
                            G = gpool.tile([128, ntc * 128], dt.float16, tag="gather")
                            nidx = ntc * 128
                            nc.gpsimd.dma_gather(
                                G[:].rearrange("p (t f) -> p t f", f=128),
                                src_view,
                                idx_t[:, icol0 + th * 8:icol0 + (th + ntc) * 8],
                                nidx,
                                nidx,
                                D,
                                queue_num=(th // CHUNK_TILES) % 2,
                            )
                        S = spool.tile([128, 128], dt.float16, tag="sel")
                        nc.vector.tensor_tensor(
                            out=S[:],
                            in0=dstp_t[:, gt:gt + 1].to_broadcast([128, 128]),
                            in1=iota_t[:],
                            op=mybir.AluOpType.is_equal,
                        )
                        nc.tensor.matmul(
                            pt[:],
                            S[:],
                            G[:, cslot * 128:(cslot + 1) * 128],
                            start=(t == 0),
                            stop=(t == tw - 1),
                        )
                        th += 1
                        gt += 1
                    lo, hi = wi * 128, (wi + 1) * 128
                    if h == 0:
                        nc.vector.tensor_copy(acc[:, lo:hi], pt[:])
                    else:
                        nc.vector.tensor_add(acc[:, lo:hi], acc[:, lo:hi], pt[:])
                    last_touch = (h == 1) or (T_hi[wi] == 0)
                    if last_touch:
                        nc.sync.dma_start(out_d[lo:hi, :], acc[:, lo:hi])
    nc.compile()
    return nc


def kernel(x, edge_index):
    global LAST_RESULT
    _ensure_ntff_hook()
    from concourse.bass_utils import run_bass_kernel_spmd

    per_core, T_lo, T_hi, L_lo, L_hi = _host_prep(x, edge_index)

    key = (T_lo, T_hi)
    if key not in _prog_cache:
        _prog_cache[key] = _build_program(T_lo, T_hi, L_lo, L_hi)
    nc = _prog_cache[key]

    res = run_bass_kernel_spmd(nc, per_core, core_ids=list(range(C)))
    LAST_RESULT = res
    out = np.concatenate([r["out"][:NLOC] for r in res.results], axis=0)
    return out.astype(np.float32)



# revision 7
# speedup vs baseline: 5.5110x; 1.0141x over previous
"""GNN message passing (gather + segment-sum) on 8 TRN2 NeuronCores.

Strategy (edge-parallel with node-partitioned output; no collectives):
  - Host: bucket edges by (core = dst // 6250, src-half, dst-window-of-128).
    Core c owns output rows [c*6250, (c+1)*6250) so partial sums ARE final --
    no all-reduce needed.  Within a core, edges are grouped by 128-node dst
    windows; each group is padded to a multiple of 128 edges (common tile
    counts across all 8 cores so one SPMD program serves every core).
  - Device, per core:
      * bulk `dma_gather` of x[src] rows (fp16 table, 256B/row) from HBM into
        SBUF, in big chunks (HW-accelerated SWDGE gather; int16 indices, so
        the table is addressed as two halves: rows [0,32768) and [32768,50000)).
      * per 128-edge tile, build one-hot S[e, n] = (dst_local[e] == n) on the
        DVE with a broadcast `is_equal` against an iota row constant.
      * matmul S^T @ G accumulated in PSUM per 128-node window: the PE does
        the segment reduction.  PSUM (f32) -> SBUF accumulator -> HBM out.
  - Host: concatenate the 8 per-core [6250, 128] slices.

The one-hot/matmul trick makes the scatter-add race-free and keeps HBM
traffic at the roofline: ~21 MB of gathered rows per core dominates.
"""

import os
import numpy as np

N = 50000          # nodes
D = 128            # feature dim
C = 8              # cores
E_TOT = 640000     # edges (any count works; hardcoded shapes only use N, D)
NLOC = N // C      # 6250 output rows per core
P = 128
N_WIN = (NLOC + P - 1) // P        # 49 windows of 128 dst nodes per core
NLOC_PAD = N_WIN * P               # 6272 (padded output rows per core)
SPLIT = 32768                      # int16 gather-index limit
SENT = 300.0                       # dst sentinel for padded edges (never matches iota 0..127)
CHUNK_TILES = 7                    # 128-edge tiles per dma_gather call (512 idx = 33
                                   # ring slots/lane, so ~3 calls pipeline in the
                                   # 128-desc SWDGE ring; >=2048 idx/call overflows it)

LAST_RESULT = None                 # BassKernelResults of the most recent run (for test.py)

_prog_cache = {}


def _ensure_ntff_hook():
    """Provide antenv.axon_hooks (missing from this image) so
    run_bass_kernel_spmd(trace=True) under axon can capture NTFF profiles.
    Harmless no-op when tracing is off or pieces are unavailable."""
    import sys
    import types
    try:
        import antenv.axon_hooks  # noqa: F401
        return
    except ImportError:
        pass
    try:
        import antenv
        mod = types.ModuleType("antenv.axon_hooks")
        mod._hook = None
        mod.set_axon_ntff_profile_hook = lambda h: setattr(mod, "_hook", h)
        mod.get_axon_ntff_profile_hook = lambda: mod._hook
        sys.modules["antenv.axon_hooks"] = mod
        antenv.axon_hooks = mod
        from trn_agent_boot.trn_boot import _ntff_profile_via_ctypes
        so_path = "/opt/axon/libaxon_pjrt.so"
        if os.path.exists(so_path):
            mod.set_axon_ntff_profile_hook(_ntff_profile_via_ctypes(so_path))
    except Exception:
        pass


def _host_prep(x, edge_index):
    """Bucket + pad edges; build per-core device input arrays."""
    x = np.asarray(x, dtype=np.float32)
    ei = np.asarray(edge_index)
    src = ei[0].astype(np.int64)
    dst = ei[1].astype(np.int64)
    E = src.shape[0]

    core = dst // NLOC
    dloc = dst - core * NLOC
    win = dloc >> 7                 # dst window within core
    pcol = dloc & 127               # dst node within window
    half = (src >= SPLIT).astype(np.int64)

    # counts[c, h, w]
    counts = np.zeros((C, 2, N_WIN), np.int64)
    np.add.at(counts, (core, half, win), 1)
    # common (max-over-cores) tile counts so one SPMD program fits all cores
    T = (-(-counts // P)).max(axis=0)        # [2, N_WIN] tiles per (half, window)
    T[0] = np.maximum(T[0], 1)               # lo pass initializes every window's acc

    L = T.sum(axis=1) * P                    # padded edges per half
    tile_base = np.zeros((2, N_WIN), np.int64)
    tile_base[0, 1:] = np.cumsum(T[0])[:-1]
    tile_base[1, 1:] = np.cumsum(T[1])[:-1]

    # sort edges by (core, half, window); stable order within groups is fine
    order = np.lexsort((win, half, core))
    s_src = src[order]
    s_p = pcol[order]
    gsz = counts.reshape(-1)
    gstart = np.zeros(C * 2 * N_WIN + 1, np.int64)
    np.cumsum(gsz, out=gstart[1:])

    xh = np.ascontiguousarray(x.astype(np.float16))
    iota = np.tile(np.arange(P, dtype=np.float16)[None, :], (P, 1))

    def wrap_idx(a):  # int16 [L] -> [128, L//16] (16-part wrap, replicated x8)
        w16 = np.ascontiguousarray(a.reshape(-1, 16).T)
        return np.ascontiguousarray(np.tile(w16, (8, 1)))

    per_core = []
    for c in range(C):
        srcs = [np.zeros(L[0], np.int16), np.zeros(L[1], np.int16)]
        dstp = [np.full(L[0], SENT, np.float16), np.full(L[1], SENT, np.float16)]
        for h in range(2):
            for w in range(N_WIN):
                g = (c * 2 + h) * N_WIN + w
                a, b = gstart[g], gstart[g + 1]
                n = b - a
                if n == 0:
                    continue
                pos = tile_base[h, w] * P
                adj = 0 if h == 0 else SPLIT
                srcs[h][pos:pos + n] = (s_src[a:b] - adj).astype(np.int16)
                dstp[h][pos:pos + n] = s_p[a:b].astype(np.float16)
        dstp_all = np.concatenate(dstp)                       # [L0 + L1]
        dstp_tile = np.ascontiguousarray(dstp_all.reshape(-1, P).T)  # [128, T_tot]
        meta = np.concatenate([dstp_tile, iota], axis=1)      # [128, T_tot + 128]
        idx_all = np.concatenate(srcs)                        # [L0 + L1]
        m = {
            "xh": xh,
            "idx": wrap_idx(idx_all),
            "meta": np.ascontiguousarray(meta),
        }
        per_core.append(m)

    return per_core, tuple(T[0]), tuple(T[1]), int(L[0]), int(L[1])


def _build_program(T_lo, T_hi, L_lo, L_hi):
    import concourse.bass as bass
    import concourse.tile as tile
    import concourse.mybir as mybir
    from concourse import bacc

    dt = mybir.dt
    nc = bacc.Bacc("TRN2", target_bir_lowering=False, debug=False, num_devices=C,
                   num_swdge_queues=4)

    xh = nc.dram_tensor("xh", [N, D], dt.float16, kind="ExternalInput")
    L_tot = L_lo + L_hi
    idx_d = nc.dram_tensor("idx", [128, L_tot // 16], dt.int16, kind="ExternalInput")
    T_tot = L_tot // P
    meta_d = nc.dram_tensor("meta", [128, T_tot + 128], dt.float16, kind="ExternalInput")
    out_d = nc.dram_tensor("out", [NLOC_PAD, D], dt.float32, kind="ExternalOutput")

    with tile.TileContext(nc) as tc:
        with (
            tc.tile_pool(name="metap", bufs=1) as metap,
            tc.tile_pool(name="gp", bufs=3) as gpool,
            tc.tile_pool(name="sp", bufs=6) as spool,
            tc.tile_pool(name="pp", bufs=4, space="PSUM") as ppool,
            tc.tile_pool(name="accp", bufs=1) as accp,
        ):
            idx_t = metap.tile([128, L_tot // 16], dt.int16, tag="idx", name="idx_t")
            nc.sync.dma_start(idx_t[:], idx_d[:])
            meta_t = metap.tile([128, T_tot + 128], dt.float16, tag="meta", name="meta_t")
            nc.sync.dma_start(meta_t[:], meta_d[:])
            dstp_t = meta_t[:, :T_tot]
            iota_t = meta_t[:, T_tot:]

            acc = accp.tile([128, N_WIN * P], dt.float32, tag="acc")

            gt = 0  # global tile index (column into dstp_t)
            for h in range(2):
                Th = T_lo if h == 0 else T_hi
                total_tiles = sum(Th)
                if total_tiles == 0:
                    continue
                src_view = xh[:SPLIT] if h == 0 else xh[SPLIT:]
                icol0 = 0 if h == 0 else L_lo // 16   # column base into idx_t
                th = 0        # tile index within this half
                G = None
                ntc = 0       # tiles in current chunk
                for wi in range(N_WIN):
                    tw = Th[wi]
                    if tw == 0:
                        continue
                    pt = ppool.tile([128, 128], dt.float32, tag="psum")
                    for t in range(tw):
                        cslot = th % CHUNK_TILES
                        if cslot == 0:
                            ntc = min(CHUNK_TILES, total_tiles - th)
                            G = gpool.tile([128, ntc * 128], dt.float16, tag="gather")
                            nidx = ntc * 128
                            nc.gpsimd.dma_gather(
                                G[:].rearrange("p (t f) -> p t f", f=128),
                                src_view,
                                idx_t[:, icol0 + th * 8:icol0 + (th + ntc) * 8],
                                nidx,
                                nidx,
                                D,
                                queue_num=(th // CHUNK_TILES) % 4,
                            )
                        S = spool.tile([128, 128], dt.float16, tag="sel")
                        nc.vector.tensor_tensor(
                            out=S[:],
                            in0=dstp_t[:, gt:gt + 1].to_broadcast([128, 128]),
                            in1=iota_t[:],
                            op=mybir.AluOpType.is_equal,
                        )
                        nc.tensor.matmul(
                            pt[:],
                            S[:],
                            G[:, cslot * 128:(cslot + 1) * 128],
                            start=(t == 0),
                            stop=(t == tw - 1),
                        )
                        th += 1
                        gt += 1
                    lo, hi = wi * 128, (wi + 1) * 128
                    if h == 0:
                        nc.vector.tensor_copy(acc[:, lo:hi], pt[:])
                    else:
                        nc.vector.tensor_add(acc[:, lo:hi], acc[:, lo:hi], pt[:])
                    last_touch = (h == 1) or (T_hi[wi] == 0)
                    if last_touch:
                        nc.sync.dma_start(out_d[lo:hi, :], acc[:, lo:hi])
    nc.compile()
    return nc


def kernel(x, edge_index):
    global LAST_RESULT
    _ensure_ntff_hook()
    from concourse.bass_utils import run_bass_kernel_spmd

    per_core, T_lo, T_hi, L_lo, L_hi = _host_prep(x, edge_index)

    key = (T_lo, T_hi)
    if key not in _prog_cache:
        _prog_cache[key] = _build_program(T_lo, T_hi, L_lo, L_hi)
    nc = _prog_cache[key]

    res = run_bass_kernel_spmd(nc, per_core, core_ids=list(range(C)))
    LAST_RESULT = res
    out = np.concatenate([r["out"][:NLOC] for r in res.results], axis=0)
    return out.astype(np.float32)



# revision 8
# speedup vs baseline: 5.6615x; 1.0273x over previous
"""GNN message passing (gather + segment-sum) on 8 TRN2 NeuronCores.

Strategy (edge-parallel with node-partitioned output; no collectives):
  - Host: bucket edges by (core = dst // 6250, src-half, dst-window-of-128).
    Core c owns output rows [c*6250, (c+1)*6250) so partial sums ARE final --
    no all-reduce needed.  Within a core, edges are grouped by 128-node dst
    windows; each group is padded to a multiple of 128 edges (common tile
    counts across all 8 cores so one SPMD program serves every core).
  - Device, per core:
      * bulk `dma_gather` of x[src] rows (fp16 table, 256B/row) from HBM into
        SBUF, in big chunks (HW-accelerated SWDGE gather; int16 indices, so
        the table is addressed as two halves: rows [0,32768) and [32768,50000)).
      * per 128-edge tile, build one-hot S[e, n] = (dst_local[e] == n) on the
        DVE with a broadcast `is_equal` against an iota row constant.
      * matmul S^T @ G accumulated in PSUM per 128-node window: the PE does
        the segment reduction.  PSUM (f32) -> SBUF accumulator -> HBM out.
  - Host: concatenate the 8 per-core [6250, 128] slices.

The one-hot/matmul trick makes the scatter-add race-free and keeps HBM
traffic at the roofline: ~21 MB of gathered rows per core dominates.
"""

import os
import numpy as np

N = 50000          # nodes
D = 128            # feature dim
C = 8              # cores
E_TOT = 640000     # edges (any count works; hardcoded shapes only use N, D)
NLOC = N // C      # 6250 output rows per core
P = 128
N_WIN = (NLOC + P - 1) // P        # 49 windows of 128 dst nodes per core
NLOC_PAD = N_WIN * P               # 6272 (padded output rows per core)
SPLIT = 32768                      # int16 gather-index limit
SENT = 300.0                       # dst sentinel for padded edges (never matches iota 0..127)
CHUNK_TILES = 8                    # 128-edge tiles per dma_gather call (512 idx = 33
                                   # ring slots/lane, so ~3 calls pipeline in the
                                   # 128-desc SWDGE ring; >=2048 idx/call overflows it)

LAST_RESULT = None                 # BassKernelResults of the most recent run (for test.py)

_prog_cache = {}


def _ensure_ntff_hook():
    """Provide antenv.axon_hooks (missing from this image) so
    run_bass_kernel_spmd(trace=True) under axon can capture NTFF profiles.
    Harmless no-op when tracing is off or pieces are unavailable."""
    import sys
    import types
    try:
        import antenv.axon_hooks  # noqa: F401
        return
    except ImportError:
        pass
    try:
        import antenv
        mod = types.ModuleType("antenv.axon_hooks")
        mod._hook = None
        mod.set_axon_ntff_profile_hook = lambda h: setattr(mod, "_hook", h)
        mod.get_axon_ntff_profile_hook = lambda: mod._hook
        sys.modules["antenv.axon_hooks"] = mod
        antenv.axon_hooks = mod
        from trn_agent_boot.trn_boot import _ntff_profile_via_ctypes
        so_path = "/opt/axon/libaxon_pjrt.so"
        if os.path.exists(so_path):
            mod.set_axon_ntff_profile_hook(_ntff_profile_via_ctypes(so_path))
    except Exception:
        pass


def _host_prep(x, edge_index):
    """Bucket + pad edges; build per-core device input arrays."""
    x = np.asarray(x, dtype=np.float32)
    ei = np.asarray(edge_index)
    src = ei[0].astype(np.int64)
    dst = ei[1].astype(np.int64)
    E = src.shape[0]

    core = dst // NLOC
    dloc = dst - core * NLOC
    win = dloc >> 7                 # dst window within core
    pcol = dloc & 127               # dst node within window
    half = (src >= SPLIT).astype(np.int64)

    # counts[c, h, w]
    counts = np.zeros((C, 2, N_WIN), np.int64)
    np.add.at(counts, (core, half, win), 1)
    # common (max-over-cores) tile counts so one SPMD program fits all cores
    T = (-(-counts // P)).max(axis=0)        # [2, N_WIN] tiles per (half, window)
    T[0] = np.maximum(T[0], 1)               # lo pass initializes every window's acc

    L = T.sum(axis=1) * P                    # padded edges per half
    tile_base = np.zeros((2, N_WIN), np.int64)
    tile_base[0, 1:] = np.cumsum(T[0])[:-1]
    tile_base[1, 1:] = np.cumsum(T[1])[:-1]

    # sort edges by (core, half, window); stable order within groups is fine
    order = np.lexsort((win, half, core))
    s_src = src[order]
    s_p = pcol[order]
    gsz = counts.reshape(-1)
    gstart = np.zeros(C * 2 * N_WIN + 1, np.int64)
    np.cumsum(gsz, out=gstart[1:])

    xh = np.ascontiguousarray(x.astype(np.float16))
    iota = np.tile(np.arange(P, dtype=np.float16)[None, :], (P, 1))

    def wrap_idx(a):  # int16 [L] -> [128, L//16] (16-part wrap, replicated x8)
        w16 = np.ascontiguousarray(a.reshape(-1, 16).T)
        return np.ascontiguousarray(np.tile(w16, (8, 1)))

    per_core = []
    for c in range(C):
        srcs = [np.zeros(L[0], np.int16), np.zeros(L[1], np.int16)]
        dstp = [np.full(L[0], SENT, np.float16), np.full(L[1], SENT, np.float16)]
        for h in range(2):
            for w in range(N_WIN):
                g = (c * 2 + h) * N_WIN + w
                a, b = gstart[g], gstart[g + 1]
                n = b - a
                if n == 0:
                    continue
                pos = tile_base[h, w] * P
                adj = 0 if h == 0 else SPLIT
                srcs[h][pos:pos + n] = (s_src[a:b] - adj).astype(np.int16)
                dstp[h][pos:pos + n] = s_p[a:b].astype(np.float16)
        dstp_all = np.concatenate(dstp)                       # [L0 + L1]
        dstp_tile = np.ascontiguousarray(dstp_all.reshape(-1, P).T)  # [128, T_tot]
        meta = np.concatenate([dstp_tile, iota], axis=1)      # [128, T_tot + 128]
        idx_all = np.concatenate(srcs)                        # [L0 + L1]
        m = {
            "xh": xh,
            "idx": wrap_idx(idx_all),
            "meta": np.ascontiguousarray(meta),
        }
        per_core.append(m)

    return per_core, tuple(T[0]), tuple(T[1]), int(L[0]), int(L[1])


def _build_program(T_lo, T_hi, L_lo, L_hi):
    import concourse.bass as bass
    import concourse.tile as tile
    import concourse.mybir as mybir
    from concourse import bacc

    dt = mybir.dt
    nc = bacc.Bacc("TRN2", target_bir_lowering=False, debug=False, num_devices=C,
                   num_swdge_queues=4)

    xh = nc.dram_tensor("xh", [N, D], dt.float16, kind="ExternalInput")
    L_tot = L_lo + L_hi
    idx_d = nc.dram_tensor("idx", [128, L_tot // 16], dt.int16, kind="ExternalInput")
    T_tot = L_tot // P
    meta_d = nc.dram_tensor("meta", [128, T_tot + 128], dt.float16, kind="ExternalInput")
    out_d = nc.dram_tensor("out", [NLOC_PAD, D], dt.float32, kind="ExternalOutput")

    with tile.TileContext(nc) as tc:
        with (
            tc.tile_pool(name="metap", bufs=1) as metap,
            tc.tile_pool(name="gp", bufs=3) as gpool,
            tc.tile_pool(name="sp", bufs=6) as spool,
            tc.tile_pool(name="pp", bufs=4, space="PSUM") as ppool,
            tc.tile_pool(name="accp", bufs=1) as accp,
        ):
            idx_t = metap.tile([128, L_tot // 16], dt.int16, tag="idx", name="idx_t")
            nc.sync.dma_start(idx_t[:], idx_d[:])
            meta_t = metap.tile([128, T_tot + 128], dt.float16, tag="meta", name="meta_t")
            nc.sync.dma_start(meta_t[:], meta_d[:])
            dstp_t = meta_t[:, :T_tot]
            iota_t = meta_t[:, T_tot:]
            iota3 = iota_t.rearrange("p (a f) -> p a f", a=1)

            acc = accp.tile([128, N_WIN * P], dt.float32, tag="acc")

            gt = 0  # global tile index (column into dstp_t)
            S4 = None
            for h in range(2):
                Th = T_lo if h == 0 else T_hi
                total_tiles = sum(Th)
                if total_tiles == 0:
                    continue
                src_view = xh[:SPLIT] if h == 0 else xh[SPLIT:]
                icol0 = 0 if h == 0 else L_lo // 16   # column base into idx_t
                th = 0        # tile index within this half
                G = None
                ntc = 0       # tiles in current chunk
                for wi in range(N_WIN):
                    tw = Th[wi]
                    if tw == 0:
                        continue
                    pt = ppool.tile([128, 128], dt.float32, tag="psum")
                    for t in range(tw):
                        cslot = th % CHUNK_TILES
                        if cslot == 0:
                            ntc = min(CHUNK_TILES, total_tiles - th)
                            G = gpool.tile([128, ntc * 128], dt.float16, tag="gather")
                            nidx = ntc * 128
                            nc.gpsimd.dma_gather(
                                G[:].rearrange("p (t f) -> p t f", f=128),
                                src_view,
                                idx_t[:, icol0 + th * 8:icol0 + (th + ntc) * 8],
                                nidx,
                                nidx,
                                D,
                                queue_num=(th // CHUNK_TILES) % 4,
                            )
                        if gt % 4 == 0:
                            nb = min(4, T_tot - gt)
                            S4 = spool.tile([128, nb, 128], dt.float16, tag="sel")
                            nc.vector.tensor_tensor(
                                out=S4[:],
                                in0=dstp_t[:, gt:gt + nb].to_broadcast([128, nb, 128]),
                                in1=iota3.to_broadcast([128, nb, 128]),
                                op=mybir.AluOpType.is_equal,
                            )
                        nc.tensor.matmul(
                            pt[:],
                            S4[:, gt % 4, :],
                            G[:, cslot * 128:(cslot + 1) * 128],
                            start=(t == 0),
                            stop=(t == tw - 1),
                        )
                        th += 1
                        gt += 1
                    lo, hi = wi * 128, (wi + 1) * 128
                    if h == 0:
                        nc.vector.tensor_copy(acc[:, lo:hi], pt[:])
                    else:
                        nc.vector.tensor_add(acc[:, lo:hi], acc[:, lo:hi], pt[:])
                    last_touch = (h == 1) or (T_hi[wi] == 0)
                    if last_touch:
                        nc.sync.dma_start(out_d[lo:hi, :], acc[:, lo:hi])
    nc.compile()
    return nc


def kernel(x, edge_index):
    global LAST_RESULT
    _ensure_ntff_hook()
    from concourse.bass_utils import run_bass_kernel_spmd

    per_core, T_lo, T_hi, L_lo, L_hi = _host_prep(x, edge_index)

    key = (T_lo, T_hi)
    if key not in _prog_cache:
        _prog_cache[key] = _build_program(T_lo, T_hi, L_lo, L_hi)
    nc = _prog_cache[key]

    res = run_bass_kernel_spmd(nc, per_core, core_ids=list(range(C)))
    LAST_RESULT = res
    out = np.concatenate([r["out"][:NLOC] for r in res.results], axis=0)
    return out.astype(np.float32)



# revision 10
# speedup vs baseline: 5.7445x; 1.0147x over previous
"""GNN message passing (gather + segment-sum) on 8 TRN2 NeuronCores.

Strategy (edge-parallel with node-partitioned output; no collectives):
  - Host: bucket edges by (core = dst // 6250, src-half, dst-window-of-128).
    Core c owns output rows [c*6250, (c+1)*6250) so partial sums ARE final --
    no all-reduce needed.  Within a core, edges are grouped by 128-node dst
    windows; each group is padded to a multiple of 128 edges (common tile
    counts across all 8 cores so one SPMD program serves every core).
  - Device, per core:
      * bulk `dma_gather` of x[src] rows (fp16 table, 256B/row) from HBM into
        SBUF, in big chunks (HW-accelerated SWDGE gather; int16 indices, so
        the table is addressed as two halves: rows [0,32768) and [32768,50000)).
      * per 128-edge tile, build one-hot S[e, n] = (dst_local[e] == n) on the
        DVE with a broadcast `is_equal` against an iota row constant.
      * matmul S^T @ G accumulated in PSUM per 128-node window: the PE does
        the segment reduction.  PSUM (f32) -> SBUF accumulator -> HBM out.
  - Host: concatenate the 8 per-core [6250, 128] slices.

The one-hot/matmul trick makes the scatter-add race-free and keeps HBM
traffic at the roofline: ~21 MB of gathered rows per core dominates.
"""

import os
import numpy as np

N = 50000          # nodes
D = 128            # feature dim
C = 8              # cores
E_TOT = 640000     # edges (any count works; hardcoded shapes only use N, D)
NLOC = N // C      # 6250 output rows per core
P = 128
N_WIN = (NLOC + P - 1) // P        # 49 windows of 128 dst nodes per core
NLOC_PAD = N_WIN * P               # 6272 (padded output rows per core)
SPLIT = 32768                      # int16 gather-index limit
SENT = 300.0                       # dst sentinel for padded edges (never matches iota 0..127)
CHUNK_TILES = 8                    # 128-edge tiles per dma_gather call (512 idx = 33
                                   # ring slots/lane, so ~3 calls pipeline in the
                                   # 128-desc SWDGE ring; >=2048 idx/call overflows it)

LAST_RESULT = None                 # BassKernelResults of the most recent run (for test.py)

_prog_cache = {}


def _ensure_ntff_hook():
    """Provide antenv.axon_hooks (missing from this image) so
    run_bass_kernel_spmd(trace=True) under axon can capture NTFF profiles.
    Harmless no-op when tracing is off or pieces are unavailable."""
    import sys
    import types
    try:
        import antenv.axon_hooks  # noqa: F401
        return
    except ImportError:
        pass
    try:
        import antenv
        mod = types.ModuleType("antenv.axon_hooks")
        mod._hook = None
        mod.set_axon_ntff_profile_hook = lambda h: setattr(mod, "_hook", h)
        mod.get_axon_ntff_profile_hook = lambda: mod._hook
        sys.modules["antenv.axon_hooks"] = mod
        antenv.axon_hooks = mod
        from trn_agent_boot.trn_boot import _ntff_profile_via_ctypes
        so_path = "/opt/axon/libaxon_pjrt.so"
        if os.path.exists(so_path):
            mod.set_axon_ntff_profile_hook(_ntff_profile_via_ctypes(so_path))
    except Exception:
        pass


def _host_prep(x, edge_index):
    """Bucket + pad edges; build per-core device input arrays."""
    x = np.asarray(x, dtype=np.float32)
    ei = np.asarray(edge_index)
    src = ei[0].astype(np.int64)
    dst = ei[1].astype(np.int64)
    E = src.shape[0]

    core = dst // NLOC
    dloc = dst - core * NLOC
    win = dloc >> 7                 # dst window within core
    pcol = dloc & 127               # dst node within window
    half = (src >= SPLIT).astype(np.int64)

    # counts[c, h, w]
    counts = np.zeros((C, 2, N_WIN), np.int64)
    np.add.at(counts, (core, half, win), 1)
    # common (max-over-cores) tile counts so one SPMD program fits all cores
    T = (-(-counts // P)).max(axis=0)        # [2, N_WIN] tiles per (half, window)
    T[0] = np.maximum(T[0], 1)               # lo pass initializes every window's acc

    L = T.sum(axis=1) * P                    # padded edges per half
    tile_base = np.zeros((2, N_WIN), np.int64)
    tile_base[0, 1:] = np.cumsum(T[0])[:-1]
    tile_base[1, 1:] = np.cumsum(T[1])[:-1]

    # sort edges by (core, half, window); stable order within groups is fine
    order = np.lexsort((win, half, core))
    s_src = src[order]
    s_p = pcol[order]
    gsz = counts.reshape(-1)
    gstart = np.zeros(C * 2 * N_WIN + 1, np.int64)
    np.cumsum(gsz, out=gstart[1:])

    xh = np.ascontiguousarray(x.astype(np.float16))
    iota = np.tile(np.arange(P, dtype=np.float16)[None, :], (P, 1))

    def wrap_idx(a):  # int16 [L] -> [128, L//16] (16-part wrap, replicated x8)
        w16 = np.ascontiguousarray(a.reshape(-1, 16).T)
        return np.ascontiguousarray(np.tile(w16, (8, 1)))

    per_core = []
    for c in range(C):
        srcs = [np.zeros(L[0], np.int16), np.zeros(L[1], np.int16)]
        dstp = [np.full(L[0], SENT, np.float16), np.full(L[1], SENT, np.float16)]
        for h in range(2):
            for w in range(N_WIN):
                g = (c * 2 + h) * N_WIN + w
                a, b = gstart[g], gstart[g + 1]
                n = b - a
                if n == 0:
                    continue
                pos = tile_base[h, w] * P
                adj = 0 if h == 0 else SPLIT
                srcs[h][pos:pos + n] = (s_src[a:b] - adj).astype(np.int16)
                dstp[h][pos:pos + n] = s_p[a:b].astype(np.float16)
        dstp_all = np.concatenate(dstp)                       # [L0 + L1]
        dstp_tile = np.ascontiguousarray(dstp_all.reshape(-1, P).T)  # [128, T_tot]
        meta = np.concatenate([dstp_tile, iota], axis=1)      # [128, T_tot + 128]
        idx_all = np.concatenate(srcs)                        # [L0 + L1]
        m = {
            "xh": xh,
            "idx": wrap_idx(idx_all),
            "meta": np.ascontiguousarray(meta),
        }
        per_core.append(m)

    return per_core, tuple(T[0]), tuple(T[1]), int(L[0]), int(L[1])


def _build_program(T_lo, T_hi, L_lo, L_hi):
    import concourse.bass as bass
    import concourse.tile as tile
    import concourse.mybir as mybir
    from concourse import bacc

    dt = mybir.dt
    nc = bacc.Bacc("TRN2", target_bir_lowering=False, debug=False, num_devices=C,
                   num_swdge_queues=4)

    xh = nc.dram_tensor("xh", [N, D], dt.float16, kind="ExternalInput")
    L_tot = L_lo + L_hi
    idx_d = nc.dram_tensor("idx", [128, L_tot // 16], dt.int16, kind="ExternalInput")
    T_tot = L_tot // P
    meta_d = nc.dram_tensor("meta", [128, T_tot + 128], dt.float16, kind="ExternalInput")
    out_d = nc.dram_tensor("out", [NLOC_PAD, D], dt.float32, kind="ExternalOutput")

    with tile.TileContext(nc) as tc:
        with (
            tc.tile_pool(name="metap", bufs=1) as metap,
            tc.tile_pool(name="gp", bufs=3) as gpool,
            tc.tile_pool(name="sp", bufs=6) as spool,
            tc.tile_pool(name="pp", bufs=4, space="PSUM") as ppool,
            tc.tile_pool(name="accp", bufs=1) as accp,
        ):
            idx_t = metap.tile([128, L_tot // 16], dt.int16, tag="idx", name="idx_t")
            nc.sync.dma_start(idx_t[:], idx_d[:])
            meta_t = metap.tile([128, T_tot + 128], dt.float16, tag="meta", name="meta_t")
            nc.sync.dma_start(meta_t[:], meta_d[:])
            dstp_t = meta_t[:, :T_tot]
            iota_t = meta_t[:, T_tot:]
            iota3 = iota_t.rearrange("p (a f) -> p a f", a=1)

            acc = accp.tile([128, N_WIN * P], dt.float32, tag="acc")

            gt = 0  # global tile index (column into dstp_t)
            S4 = None
            for h in range(2):
                Th = T_lo if h == 0 else T_hi
                total_tiles = sum(Th)
                if total_tiles == 0:
                    continue
                src_view = xh[:SPLIT] if h == 0 else xh[SPLIT:]
                icol0 = 0 if h == 0 else L_lo // 16   # column base into idx_t
                th = 0        # tile index within this half
                G = None
                ntc = 0       # tiles in current chunk
                for wi in range(N_WIN):
                    tw = Th[wi]
                    if tw == 0:
                        continue
                    pt = ppool.tile([128, 128], dt.float32, tag="psum")
                    for t in range(tw):
                        cslot = th % CHUNK_TILES
                        if cslot == 0:
                            ntc = min(CHUNK_TILES, total_tiles - th)
                            G = gpool.tile([128, ntc * 128], dt.float16, tag="gather")
                            nidx = ntc * 128
                            nc.gpsimd.dma_gather(
                                G[:].rearrange("p (t f) -> p t f", f=128),
                                src_view,
                                idx_t[:, icol0 + th * 8:icol0 + (th + ntc) * 8],
                                nidx,
                                nidx,
                                D,
                                queue_num=(th // CHUNK_TILES) % 4,
                            )
                        if gt % 4 == 0:
                            nb = min(4, T_tot - gt)
                            S4 = spool.tile([128, nb, 128], dt.float16, tag="sel")
                            nc.vector.tensor_tensor(
                                out=S4[:],
                                in0=dstp_t[:, gt:gt + nb].to_broadcast([128, nb, 128]),
                                in1=iota3.to_broadcast([128, nb, 128]),
                                op=mybir.AluOpType.is_equal,
                            )
                        nc.tensor.matmul(
                            pt[:],
                            S4[:, gt % 4, :],
                            G[:, cslot * 128:(cslot + 1) * 128],
                            start=(t == 0),
                            stop=(t == tw - 1),
                        )
                        th += 1
                        gt += 1
                    lo, hi = wi * 128, (wi + 1) * 128
                    if h == 0:
                        nc.vector.tensor_copy(acc[:, lo:hi], pt[:])
                    else:
                        nc.vector.tensor_add(acc[:, lo:hi], acc[:, lo:hi], pt[:])
                    last_touch = (h == 1) or (T_hi[wi] == 0)
                    if last_touch:
                        nc.sync.dma_start(out_d[lo:hi, :], acc[:, lo:hi])
    nc.compile()
    return nc


def kernel(x, edge_index):
    global LAST_RESULT
    _ensure_ntff_hook()
    from concourse.bass_utils import run_bass_kernel_spmd

    per_core, T_lo, T_hi, L_lo, L_hi = _host_prep(x, edge_index)

    key = (T_lo, T_hi)
    if key not in _prog_cache:
        _prog_cache[key] = _build_program(T_lo, T_hi, L_lo, L_hi)
    nc = _prog_cache[key]

    res = run_bass_kernel_spmd(nc, per_core, core_ids=list(range(C)))
    LAST_RESULT = res
    out = np.concatenate([r["out"][:NLOC] for r in res.results], axis=0)
    return out.astype(np.float32)



# revision 18
# speedup vs baseline: 10.4037x; 1.8111x over previous
"""GNN message passing (gather + segment-sum) on 8 TRN2 NeuronCores.

Strategy (edge-parallel with node-partitioned output; no collectives):
  - Host: bucket edges by (core = dst // 6250, src-half, dst-window-of-128).
    Core c owns output rows [c*6250, (c+1)*6250) so partial sums ARE final --
    no all-reduce needed.  Within a core, edges are grouped by 128-node dst
    windows; each group is padded to a multiple of 128 edges (common tile
    counts across all 8 cores so one SPMD program serves every core).
  - Device, per core:
      * bulk `dma_gather` of x[src] rows (fp16 table, 256B/row) from HBM into
        SBUF, in 1024-index chunks (Q7 SWDGE gather; int16 indices, so the
        table is addressed as two halves: rows [0,32768) and [32768,50000)).
        Calls round-robin over 4 SWDGE queues: each queue has its own
        descriptor ring/carveout and Q7 core pair, so queue k+1's descriptor
        generation overlaps queue k's DMA drain + ring reclaim (the single-
        queue version serializes on reclaim and runs ~2x slower).
      * per 128-edge tile, build one-hot S[e, n] = (dst_local[e] == n) on the
        DVE with a broadcast `is_equal` against an iota row constant, batched
        8 tiles per instruction via stride-0 3D access patterns.
      * matmul S^T @ G accumulated in PSUM per 128-node window: the PE does
        the segment reduction.  PSUM (f32) -> SBUF accumulator -> HBM out.
  - Host: concatenate the 8 per-core [6250, 128] slices.

The one-hot/matmul trick makes the scatter-add race-free; the Q7 descriptor
generation (~2.3 ns/row once the 4 queues hide DMA drain/ring reclaim) is the
critical path: ~205us of the ~240us wall, plus ~20us startup (engine preamble
+ input loads + first-call Q7 icache warmup) and ~15us pipeline tail.
Gather-call pipelining needs gpool bufs >= ~6: with fewer buffers the next
call stalls on G-tile reuse and the queues cannot overlap (3 bufs -> ~445us).
"""

import os
import numpy as np

N = 50000          # nodes
D = 128            # feature dim
C = 8              # cores
E_TOT = 640000     # edges (any count works; hardcoded shapes only use N, D)
NLOC = N // C      # 6250 output rows per core
P = 128
N_WIN = (NLOC + P - 1) // P        # 49 windows of 128 dst nodes per core
NLOC_PAD = N_WIN * P               # 6272 (padded output rows per core)
SPLIT = 32768                      # int16 gather-index limit
SENT = 300.0                       # dst sentinel for padded edges (never matches iota 0..127)
CHUNK_TILES = 8                    # 128-edge tiles per dma_gather call (1024 idx).
                                   # Measured HW limits: 1024 idx/call works,
                                   # 1280+ wedges the core (SWDGE descriptor
                                   # carveout overflow, ~1024-1296 desc cap/queue)

LAST_RESULT = None                 # BassKernelResults of the most recent run (for test.py)

_prog_cache = {}


def _ensure_ntff_hook():
    """Provide antenv.axon_hooks (missing from this image) so
    run_bass_kernel_spmd(trace=True) under axon can capture NTFF profiles.
    Harmless no-op when tracing is off or pieces are unavailable."""
    import sys
    import types
    try:
        import antenv.axon_hooks  # noqa: F401
        return
    except ImportError:
        pass
    try:
        import antenv
        mod = types.ModuleType("antenv.axon_hooks")
        mod._hook = None
        mod.set_axon_ntff_profile_hook = lambda h: setattr(mod, "_hook", h)
        mod.get_axon_ntff_profile_hook = lambda: mod._hook
        sys.modules["antenv.axon_hooks"] = mod
        antenv.axon_hooks = mod
        from trn_agent_boot.trn_boot import _ntff_profile_via_ctypes
        so_path = "/opt/axon/libaxon_pjrt.so"
        if os.path.exists(so_path):
            mod.set_axon_ntff_profile_hook(_ntff_profile_via_ctypes(so_path))
    except Exception:
        pass


def _host_prep(x, edge_index):
    """Bucket + pad edges; build per-core device input arrays."""
    x = np.asarray(x, dtype=np.float32)
    ei = np.asarray(edge_index)
    src = ei[0].astype(np.int64)
    dst = ei[1].astype(np.int64)
    E = src.shape[0]

    core = dst // NLOC
    dloc = dst - core * NLOC
    win = dloc >> 7                 # dst window within core
    pcol = dloc & 127               # dst node within window
    half = (src >= SPLIT).astype(np.int64)

    # counts[c, h, w]
    counts = np.zeros((C, 2, N_WIN), np.int64)
    np.add.at(counts, (core, half, win), 1)
    # common (max-over-cores) tile counts so one SPMD program fits all cores
    T = (-(-counts // P)).max(axis=0)        # [2, N_WIN] tiles per (half, window)
    T[0] = np.maximum(T[0], 1)               # lo pass initializes every window's acc

    L = T.sum(axis=1) * P                    # padded edges per half
    tile_base = np.zeros((2, N_WIN), np.int64)
    tile_base[0, 1:] = np.cumsum(T[0])[:-1]
    tile_base[1, 1:] = np.cumsum(T[1])[:-1]

    # sort edges by (core, half, window); stable order within groups is fine
    order = np.lexsort((win, half, core))
    s_src = src[order]
    s_p = pcol[order]
    gsz = counts.reshape(-1)
    gstart = np.zeros(C * 2 * N_WIN + 1, np.int64)
    np.cumsum(gsz, out=gstart[1:])

    xh = np.ascontiguousarray(x.astype(np.float16))
    iota = np.tile(np.arange(P, dtype=np.float16)[None, :], (P, 1))

    def wrap_idx(a):  # int16 [L] -> [128, L//16] (16-part wrap, replicated x8)
        w16 = np.ascontiguousarray(a.reshape(-1, 16).T)
        return np.ascontiguousarray(np.tile(w16, (8, 1)))

    per_core = []
    for c in range(C):
        srcs = [np.zeros(L[0], np.int16), np.zeros(L[1], np.int16)]
        dstp = [np.full(L[0], SENT, np.float16), np.full(L[1], SENT, np.float16)]
        for h in range(2):
            for w in range(N_WIN):
                g = (c * 2 + h) * N_WIN + w
                a, b = gstart[g], gstart[g + 1]
                n = b - a
                if n == 0:
                    continue
                pos = tile_base[h, w] * P
                adj = 0 if h == 0 else SPLIT
                srcs[h][pos:pos + n] = (s_src[a:b] - adj).astype(np.int16)
                dstp[h][pos:pos + n] = s_p[a:b].astype(np.float16)
        dstp_all = np.concatenate(dstp)                       # [L0 + L1]
        dstp_tile = np.ascontiguousarray(dstp_all.reshape(-1, P).T)  # [128, T_tot]
        meta = np.concatenate([dstp_tile, iota], axis=1)      # [128, T_tot + 128]
        idx_all = np.concatenate(srcs)                        # [L0 + L1]
        m = {
            "xh": xh,
            "idx": wrap_idx(idx_all),
            "meta": np.ascontiguousarray(meta),
        }
        per_core.append(m)

    return per_core, tuple(T[0]), tuple(T[1]), int(L[0]), int(L[1])


def _build_program(T_lo, T_hi, L_lo, L_hi):
    import concourse.bass as bass
    import concourse.tile as tile
    import concourse.mybir as mybir
    from concourse import bacc

    dt = mybir.dt
    nc = bacc.Bacc("TRN2", target_bir_lowering=False, debug=False, num_devices=C,
                   num_swdge_queues=4)

    xh = nc.dram_tensor("xh", [N, D], dt.float16, kind="ExternalInput")
    L_tot = L_lo + L_hi
    idx_d = nc.dram_tensor("idx", [128, L_tot // 16], dt.int16, kind="ExternalInput")
    T_tot = L_tot // P
    meta_d = nc.dram_tensor("meta", [128, T_tot + 128], dt.float16, kind="ExternalInput")
    out_d = nc.dram_tensor("out", [NLOC_PAD, D], dt.float32, kind="ExternalOutput")

    with tile.TileContext(nc) as tc:
        with (
            tc.tile_pool(name="metap", bufs=1) as metap,
            tc.tile_pool(name="gp", bufs=3) as gpool,
            tc.tile_pool(name="sp", bufs=6) as spool,
            tc.tile_pool(name="pp", bufs=6, space="PSUM") as ppool,
            tc.tile_pool(name="accp", bufs=1) as accp,
        ):
            idx_t = metap.tile([128, L_tot // 16], dt.int16, tag="idx", name="idx_t")
            nc.sync.dma_start(idx_t[:], idx_d[:])
            meta_t = metap.tile([128, T_tot + 128], dt.float16, tag="meta", name="meta_t")
            nc.sync.dma_start(meta_t[:], meta_d[:])
            dstp_t = meta_t[:, :T_tot]
            iota_t = meta_t[:, T_tot:]
            iota3 = iota_t.rearrange("p (a f) -> p a f", a=1)

            acc = accp.tile([128, N_WIN * P], dt.float32, tag="acc")

            gt = 0  # global tile index (column into dstp_t)
            S4 = None
            for h in range(2):
                Th = T_lo if h == 0 else T_hi
                total_tiles = sum(Th)
                if total_tiles == 0:
                    continue
                src_view = xh[:SPLIT] if h == 0 else xh[SPLIT:]
                icol0 = 0 if h == 0 else L_lo // 16   # column base into idx_t
                th = 0        # tile index within this half
                G = None
                ntc = 0       # tiles in current chunk
                for wi in range(N_WIN):
                    tw = Th[wi]
                    if tw == 0:
                        continue
                    pt = ppool.tile([128, 128], dt.float32, tag="psum")
                    for t in range(tw):
                        cslot = th % CHUNK_TILES
                        if cslot == 0:
                            ntc = min(CHUNK_TILES, total_tiles - th)
                            G = gpool.tile([128, ntc * 128], dt.float16, tag="gather")
                            nidx = ntc * 128
                            nc.gpsimd.dma_gather(
                                G[:].rearrange("p (t f) -> p t f", f=128),
                                src_view,
                                idx_t[:, icol0 + th * 8:icol0 + (th + ntc) * 8],
                                nidx,
                                nidx,
                                D,
                                queue_num=(th // CHUNK_TILES) % 4,
                            )
                        if gt % 8 == 0:
                            nb = min(8, T_tot - gt)
                            S4 = spool.tile([128, nb, 128], dt.float16, tag="sel")
                            nc.vector.tensor_tensor(
                                out=S4[:],
                                in0=dstp_t[:, gt:gt + nb].to_broadcast([128, nb, 128]),
                                in1=iota3.to_broadcast([128, nb, 128]),
                                op=mybir.AluOpType.is_equal,
                            )
                        nc.tensor.matmul(
                            pt[:],
                            S4[:, gt % 8, :],
                            G[:, cslot * 128:(cslot + 1) * 128],
                            start=(t == 0),
                            stop=(t == tw - 1),
                        )
                        th += 1
                        gt += 1
                    lo, hi = wi * 128, (wi + 1) * 128
                    if h == 0:
                        nc.vector.tensor_copy(acc[:, lo:hi], pt[:])
                    else:
                        nc.vector.tensor_add(acc[:, lo:hi], acc[:, lo:hi], pt[:])
                    last_touch = (h == 1) or (T_hi[wi] == 0)
                    if last_touch:
                        nc.sync.dma_start(out_d[lo:hi, :], acc[:, lo:hi])
    nc.compile()
    return nc


def kernel(x, edge_index):
    global LAST_RESULT
    _ensure_ntff_hook()
    from concourse.bass_utils import run_bass_kernel_spmd

    per_core, T_lo, T_hi, L_lo, L_hi = _host_prep(x, edge_index)

    key = (T_lo, T_hi)
    if key not in _prog_cache:
        _prog_cache[key] = _build_program(T_lo, T_hi, L_lo, L_hi)
    nc = _prog_cache[key]

    res = run_bass_kernel_spmd(nc, per_core, core_ids=list(range(C)))
    LAST_RESULT = res
    out = np.concatenate([r["out"][:NLOC] for r in res.results], axis=0)
    return out.astype(np.float32)

